# revision 16
# baseline (speedup 1.0000x reference)
"""Trainium2 Bass kernel for a LLaMA-style causal attention block.

Sharding (8 NeuronCores, one trn2 chip):
  - Tensor-parallel over heads: core c owns heads [4c, 4c+4) -> wq/wk/wv column
    slices [4096, 512]; computes qT/kT/v + RoPE + causal attention for its heads.
  - attnT [512, 2048] (bf16) is AllGather'd per sq quarter -> each core computes
    out[:, 512c:512c+512] = attn @ wo_cols.  Host concatenates column slices.

Layout trick: everything is computed transposed ([head_dim, seq]) so no
on-device transposes are needed:
  qT/kT = w_h.T @ xT      (xT host-pretransposed)
  scoresT[sk, sq] = kT_tile.T @ qT
  attnT[hd, sq] = v_tile.T @ expT
  out[sq, cols] = attnT_full_tile.T @ wo_tile
exp() needs no max-subtraction: scores are O(1) by construction.

v3 structure (vs v2):
  - trace showed PE at 92% busy but clocked 13/16 (GPIO power throttle) with
    three gaps (strip0 PSUM waits, tail AllGather exposure) each also causing
    a cold 4/8 re-throttle.  All v3 changes remove PE idle or PE cycles:
  - wk preloaded at prologue on the ACT queue (pass B was DMA-starved).
  - strip 0: pass B accumulates in the attention PSUM pool (idle in strip 0)
    so it starts while pass A's rope evacuations drain -> no PSUM-wait gap.
  - softmax denominator: ALL exp blocks accumulated on DVE into an f32 tile,
    ONE ones-matmul per (head, quarter) (v2 ran 5 ones-matmuls = 41k wasted
    PE cycles).  Mask adds trimmed to the 128 columns that are actually
    masked.
  - epilogue: outproj q0-q3 emitted as one primary stream; attention of the
    last quarter drains inside the first quarter (frac 0.25) so its
    AllGather (~47us incl. cross-core skew) completes long before outproj q3
    needs it.  q3's last 4 contraction tiles run ss-major with per-ss stores
    to stagger the final store tail.

Compute dtype bf16 (f32 PSUM accumulation), I/O f32.
"""

import math
import os
import sys

for _p in ("/opt/trn_rl_repo",):
    if os.path.isdir(_p) and _p not in sys.path:
        sys.path.insert(0, _p)

import numpy as np
import ml_dtypes

N_CORES = 8
B, S, D, H = 1, 2048, 4096, 32
HD = D // H          # 128
HPC = H // N_CORES   # 4 heads per core
CW = D // N_CORES    # 512 columns per core
NK = D // 128        # 32 contraction tiles
SQT = 512            # sq tile width
NSQ = S // SQT       # 4
SCALE = 1.0 / math.sqrt(HD)

_CACHE = {}
LAST_RESULT = None   # test harness reads exec_time_ns from here


def _build():
    import concourse.mybir as mybir
    import concourse.tile as tile
    from concourse import bacc
    from concourse.bass_isa import ReduceOp

    dt = mybir.dt
    f32, bf16 = dt.float32, dt.bfloat16

    nc = bacc.Bacc("TRN2", target_bir_lowering=False, debug=False,
                   num_devices=N_CORES)

    xT = nc.dram_tensor("xT", [D, S], bf16, kind="ExternalInput").ap()
    wq = nc.dram_tensor("wq", [D, CW], bf16, kind="ExternalInput").ap()
    wk = nc.dram_tensor("wk", [D, CW], bf16, kind="ExternalInput").ap()
    wv = nc.dram_tensor("wv", [D, CW], bf16, kind="ExternalInput").ap()
    wo = nc.dram_tensor("wo", [D, CW], bf16, kind="ExternalInput").ap()
    cosT = nc.dram_tensor("cosT", [HD, S], bf16, kind="ExternalInput").ap()
    sinT = nc.dram_tensor("sinT", [HD, S], bf16, kind="ExternalInput").ap()
    ones = nc.dram_tensor("ones", [HD, 1], bf16, kind="ExternalInput").ap()
    masks = nc.dram_tensor("masks", [4, 128, SQT], bf16, kind="ExternalInput").ap()
    out = nc.dram_tensor("out", [S, CW], bf16, kind="ExternalOutput").ap()

    swap_mask = []
    for i in range(16):
        swap_mask += [2 * i + 1, 2 * i]

    rg = [list(range(N_CORES))]

    with tile.TileContext(nc) as tc:
        with (
            tc.tile_pool(name="consts", bufs=1) as cpool,
            tc.tile_pool(name="wqp", bufs=NK) as wqp,    # wq resident; reused by wo
            tc.tile_pool(name="wkp", bufs=NK) as wkp,    # wk resident; reused by ag
            tc.tile_pool(name="wvp", bufs=NK) as wvp,    # wv resident
            tc.tile_pool(name="xp", bufs=33) as xpool,   # x strip ring
            tc.tile_pool(name="res", bufs=1) as res,     # qrot/krot/v_sb
            tc.tile_pool(name="rope", bufs=2) as ropep,
            tc.tile_pool(name="expp", bufs=4) as expp,
            tc.tile_pool(name="accp", bufs=1) as accp,
            tc.tile_pool(name="nrm", bufs=1) as nrm,
            tc.tile_pool(name="attnsb", bufs=2) as attnsb,
            tc.tile_pool(name="psq", bufs=4, space="PSUM") as psq,
            tc.tile_pool(name="psa", bufs=4, space="PSUM") as psa,
            tc.tile_pool(name="dram", bufs=1, space="DRAM") as dram,
        ):
            # resident results of QKV+rope
            qrot = [res.tile([HD, S], bf16, name=f"qrot{h}") for h in range(HPC)]
            krot = [res.tile([HD, S], bf16, name=f"krot{h}") for h in range(HPC)]
            v_sb = [res.tile([128, CW], bf16, name=f"v{i}") for i in range(S // 128)]

            # AllGather bounce buffers (one per sq quarter)
            ag_in = [dram.tile([HPC * HD, SQT], bf16, name=f"agin{q}")
                     for q in range(NSQ)]
            ag_out = [dram.tile([D, SQT], bf16, addr_space="Shared",
                                name=f"agout{q}") for q in range(NSQ)]

            cos_sb = cpool.tile([HD, S], bf16, name="cos_sb")
            sin_sb = cpool.tile([HD, S], bf16, name="sin_sb")
            ones_sb = cpool.tile([HD, 1], bf16, name="ones_sb")
            mask_sb = [cpool.tile([128, SQT], bf16, name=f"mask{r}")
                       for r in range(4)]

            # ---- resident weights, loaded once on parallel queues ----
            # (only SP/Activation/gpsimd can issue DMAs.  HBM demand in the
            # first ~45us is the constraint: wq+x must land for pass A, so
            # wq rides gpsimd and x strip 0 splits across SP/Activation.
            # Everything else queues BEHIND those transfers on the same DMA
            # queues so the hardware self-staggers it: scalar queue carries
            # x-odds -> consts -> wk (wk needed from ~50us), gpsimd carries
            # wq -> wv (wv needed from ~80us).)
            wq_sb, wk_sb, wv_sb = [], [], []
            for d in range(NK):
                wqt = wqp.tile([128, CW], bf16, tag="wq", name=f"wq{d}")
                nc.gpsimd.dma_start(wqt[:], wq[d * 128:(d + 1) * 128, :])
                wq_sb.append(wqt)

            wo_sb = []   # filled during strip 3 (reuses wqp slots)

            def emit_rope(pst, rot, sq0):
                # rot = t*cos + shuffle(t)*sin'   (sin' sign-baked)
                tbf = ropep.tile([128, SQT], bf16, tag="rbf", name="rbf")
                nc.scalar.copy(tbf[:], pst[:])          # frees the PSUM bank
                tsw = ropep.tile([128, SQT], bf16, tag="rsw", name="rsw")
                nc.vector.stream_shuffle(tsw[:], tbf[:], swap_mask)
                nc.vector.tensor_mul(tbf[:], tbf[:], cos_sb[:, sq0:sq0 + SQT])
                nc.vector.tensor_mul(tsw[:], tsw[:], sin_sb[:, sq0:sq0 + SQT])
                nc.vector.tensor_add(rot[:, sq0:sq0 + SQT], tbf[:], tsw[:])

            def strip_units(st):
                """Strip st as a list of emit units (~1us of PE work each):
                A: q heads 0-3 (streams x+wq), B: k heads 0-3, C: v. One
                PSUM bank per accumulator; in strip 0 pass B borrows the
                (idle) attention pool so it starts while pass A's rope
                evacuations drain; elsewhere interleaved attention work
                gives the ACT engine time to free banks between passes."""
                sq0 = st * SQT
                units = []
                x_tiles = []
                st_state = {}

                def u_head():
                    for d in range(NK):
                        xt = xpool.tile([128, SQT], bf16, tag="x",
                                        name=f"x{st}_{d}")
                        eng = nc.scalar if (st == 0 and d % 2) else nc.sync
                        eng.dma_start(xt[:], xT[d * 128:(d + 1) * 128,
                                                sq0:sq0 + SQT])
                        x_tiles.append(xt)
                    if st == 0:
                        # per-DMA-queue bandwidth is ~100 GB/s under full
                        # contention, so wk (needed from ~50us) splits across
                        # the scalar and SP queues, each of which
                        # self-staggers it behind the x-strip transfers.
                        # consts are only needed by the rope muls (~55us+)
                        # and wv by pass C (~82us+), so they queue last.
                        for d in range(NK):
                            wkt = wkp.tile([128, CW], bf16, tag="wk",
                                           name=f"wk{d}")
                            eng = nc.scalar if d % 2 == 0 else nc.sync
                            eng.dma_start(wkt[:],
                                          wk[d * 128:(d + 1) * 128, :])
                            wk_sb.append(wkt)
                        nc.scalar.dma_start(cos_sb[:], cosT[:])
                        nc.scalar.dma_start(sin_sb[:], sinT[:])
                        nc.scalar.dma_start(ones_sb[:], ones[:])
                        for r in range(4):
                            nc.scalar.dma_start(mask_sb[r][:], masks[r])
                        for d in range(NK):
                            wvt = wvp.tile([128, CW], bf16, tag="wv",
                                           name=f"wv{d}")
                            nc.sync.dma_start(wvt[:],
                                              wv[d * 128:(d + 1) * 128, :])
                            wv_sb.append(wvt)
                units.append(u_head)

                for pi, wname in enumerate(("wq", "wk")):
                    def u_pre(pi=pi):
                        if st == 0 and pi == 1:
                            pool, tg = psa, "c"
                        else:
                            pool, tg = psq, "b"
                        st_state[pi] = [pool.tile([128, SQT], f32, tag=tg,
                                                  name=f"qk{st}_{pi}_{h}")
                                        for h in range(HPC)]
                    units.append(u_pre)
                    for d in range(NK):
                        def u_d(d=d, pi=pi):
                            wsb = wq_sb if pi == 0 else wk_sb
                            first, last = d == 0, d == NK - 1
                            xt = x_tiles[d]
                            for h in range(HPC):
                                nc.tensor.matmul(
                                    st_state[pi][h][:],
                                    wsb[d][:, h * HD:(h + 1) * HD],
                                    xt[:], start=first, stop=last)
                        units.append(u_d)
                    for h in range(HPC):
                        def u_rope(h=h, pi=pi):
                            rots = qrot if pi == 0 else krot
                            emit_rope(st_state[pi][h], rots[h], sq0)
                        units.append(u_rope)
                    if st == 3 and pi == 0:
                        def u_wo():
                            for d in range(NK):
                                wot = wqp.tile([128, CW], bf16, tag="wq",
                                               name=f"wo{d}")
                                nc.gpsimd.dma_start(
                                    wot[:], wo[d * 128:(d + 1) * 128, :])
                                wo_sb.append(wot)
                        units.append(u_wo)

                def u_vpre():
                    st_state["v"] = [psq.tile([128, CW], f32, tag="b",
                                              name=f"vps{st}_{ss}")
                                     for ss in range(4)]
                units.append(u_vpre)
                for d in range(NK):
                    def u_vd(d=d):
                        first, last = d == 0, d == NK - 1
                        for ss in range(4):
                            nc.tensor.matmul(
                                st_state["v"][ss][:],
                                x_tiles[d][:, ss * 128:(ss + 1) * 128],
                                wv_sb[d][:], start=first, stop=last)
                    units.append(u_vd)
                def u_vcopy():
                    for ss in range(4):
                        nc.scalar.copy(v_sb[st * 4 + ss][:],
                                       st_state["v"][ss][:])
                units.append(u_vcopy)
                return units

            def attn_units(sqT):
                """Attention for quarter sqT as emit units, interleaved into
                the following strip (or the output projection for sqT=3) so
                its exp/DVE load overlaps foreign matmul streams."""
                sq0 = sqT * SQT
                nblk = 4 * (sqT + 1)
                units = []
                for h in range(HPC):
                    hs = {}
                    exp_tiles = []

                    def emit_pv(j, hs=hs, h=h, exp_tiles=exp_tiles, nblk=nblk):
                        first, last = j == 0, j == nblk - 1
                        e, off = exp_tiles[j]
                        n = SQT - off
                        nc.tensor.matmul(hs["attn"][:, off:SQT],
                                         v_sb[j][:, h * HD:(h + 1) * HD],
                                         e[:, 0:n],
                                         start=first, stop=last)

                    for i in range(nblk):
                        def u_blk(i=i, h=h, hs=hs, exp_tiles=exp_tiles,
                                  nblk=nblk, emit_pv=emit_pv):
                            if i == 0:
                                hs["attn"] = psa.tile([HD, SQT], f32, tag="c",
                                                      name=f"aps{sqT}_{h}")
                                hs["acc"] = accp.tile(
                                    [128, SQT], f32, tag="acc",
                                    name=f"acc{sqT}_{h}")
                            r = i - 4 * sqT
                            off = max(0, r) * 128
                            n = SQT - off
                            sc = psa.tile([128, SQT], f32, tag="c",
                                          name=f"sc{sqT}_{h}_{i}")
                            nc.tensor.matmul(sc[:, 0:n],
                                             krot[h][:, i * 128:(i + 1) * 128],
                                             qrot[h][:, sq0 + off:sq0 + SQT],
                                             start=True, stop=True)
                            if r >= 0:
                                # only the first 128 columns of the block can
                                # contain masked (query < key) elements
                                w = min(128, n)
                                nc.vector.tensor_add(sc[:, 0:w], sc[:, 0:w],
                                                     mask_sb[r][:, off:off + w])
                            e = expp.tile([128, SQT], bf16, tag="e",
                                          name=f"e{sqT}_{h}_{i}")
                            nc.scalar.activation(
                                e[:, 0:n], sc[:, 0:n],
                                mybir.ActivationFunctionType.Exp, scale=SCALE)
                            if i == 0:
                                nc.vector.tensor_copy(hs["acc"][:], e[:])
                            else:
                                nc.vector.tensor_add(hs["acc"][:, off:SQT],
                                                     hs["acc"][:, off:SQT],
                                                     e[:, 0:n])
                            exp_tiles.append((e, off))
                            if i >= 2:
                                emit_pv(i - 2)
                        units.append(u_blk)

                    def u_tail(h=h, hs=hs, exp_tiles=exp_tiles, nblk=nblk,
                               emit_pv=emit_pv):
                        emit_pv(nblk - 2)
                        emit_pv(nblk - 1)
                        # denominator: partition all-reduce of the exp
                        # accumulator on gpsimd (result broadcast to all
                        # partitions), replacing the ones-matmul + broadcast
                        den_bc = nrm.tile([128, SQT], f32, tag="bc",
                                          name=f"den{sqT}_{h}")
                        hs["den"] = den_bc
                        nc.gpsimd.partition_all_reduce(
                            den_bc[:], hs["acc"][:], 128, ReduceOp.add)
                    units.append(u_tail)

                    def u_norm(h=h, hs=hs):
                        rec = nrm.tile([128, SQT], f32, tag="rec",
                                       name=f"rec{sqT}_{h}")
                        nc.vector.reciprocal_approx_fast(out=rec[:],
                                                         in_=hs["den"][:])
                        araw = attnsb.tile([HD, SQT], bf16, tag="a",
                                           name=f"araw{sqT}_{h}")
                        nc.vector.tensor_copy(araw[:], hs["attn"][:])
                        a_sb = attnsb.tile([HD, SQT], bf16, tag="a",
                                           name=f"asb{sqT}_{h}")
                        nc.vector.tensor_mul(a_sb[:], araw[:], rec[:])
                        nc.gpsimd.dma_start(
                            ag_in[sqT][h * HD:(h + 1) * HD, :], a_sb[:])
                    units.append(u_norm)

                def u_ag():
                    nc.gpsimd.collective_compute(
                        "AllGather", mybir.AluOpType.bypass, replica_groups=rg,
                        ins=[ag_in[sqT].opt()], outs=[ag_out[sqT].opt()])
                units.append(u_ag)
                return units

            def outproj_units(q):
                units = []
                qs = {}
                nd_main = NK if q < 3 else NK - 4
                for d in range(nd_main):
                    def u_od(d=d, q=q, qs=qs):
                        if d == 0:
                            qs["o"] = [psq.tile([128, CW], f32, tag="b",
                                                name=f"ops{q}_{ss}")
                                       for ss in range(4)]
                        agt = wkp.tile([128, SQT], bf16, tag="wk",
                                       name=f"agt{q}_{d}")
                        eng = nc.gpsimd if d % 2 else nc.sync
                        eng.dma_start(agt[:],
                                      ag_out[q][d * 128:(d + 1) * 128, :])
                        first, last = d == 0, d == NK - 1
                        for ss in range(4):
                            nc.tensor.matmul(
                                qs["o"][ss][:],
                                agt[:, ss * 128:(ss + 1) * 128],
                                wo_sb[d][:], start=first, stop=last)
                    units.append(u_od)
                if q < 3:
                    def u_ost(q=q, qs=qs):
                        # stores ride the ACT queue: gpsimd must stay free
                        # for the attn3 norm broadcasts + ag_in DMAs
                        for ss in range(4):
                            o = attnsb.tile([128, CW], bf16, tag="a",
                                            name=f"o{q}_{ss}")
                            nc.scalar.copy(o[:], qs["o"][ss][:])
                            nc.scalar.dma_start(
                                out[q * SQT + ss * 128:q * SQT + (ss + 1) * 128,
                                    :], o[:])
                    units.append(u_ost)
                else:
                    # last 4 contraction tiles ss-major with per-ss stores so
                    # the final store tail staggers instead of arriving at once
                    def u_ldtail(qs=qs):
                        qs["agt_tail"] = []
                        for d in range(NK - 4, NK):
                            agt = wkp.tile([128, SQT], bf16, tag="wk",
                                           name=f"agt3_{d}")
                            eng = nc.gpsimd if d % 2 else nc.sync
                            eng.dma_start(
                                agt[:], ag_out[3][d * 128:(d + 1) * 128, :])
                            qs["agt_tail"].append(agt)
                    units.append(u_ldtail)
                    for ss in range(4):
                        def u_fin(ss=ss, qs=qs):
                            for j, d in enumerate(range(NK - 4, NK)):
                                nc.tensor.matmul(
                                    qs["o"][ss][:],
                                    qs["agt_tail"][j][:, ss * 128:(ss + 1) * 128],
                                    wo_sb[d][:], start=False, stop=(d == NK - 1))
                            o = attnsb.tile([128, CW], bf16, tag="a",
                                            name=f"o3_{ss}")
                            nc.scalar.copy(o[:], qs["o"][ss][:])
                            nc.sync.dma_start(
                                out[3 * SQT + ss * 128:3 * SQT + (ss + 1) * 128,
                                    :], o[:])
                        units.append(u_fin)
                return units

            def interleave(primary, secondary, frac=1.0):
                # drain `secondary` within the first `frac` of `primary`
                n, m = len(primary), len(secondary)
                j = 0
                for i, u in enumerate(primary):
                    u()
                    target = min(m, int((i + 1) * m / (n * frac)))
                    while j < target:
                        secondary[j]()
                        j += 1
                while j < m:
                    secondary[j]()
                    j += 1

            for st in range(NSQ):
                su = strip_units(st)
                au = attn_units(st - 1) if st >= 1 else []
                interleave(su, au)
            # attention of the last strip drains inside outproj q0 so its
            # AllGather (incl. cross-core skew wait) hides under q1-q2
            ou_all = []
            for q in range(NSQ):
                ou_all += outproj_units(q)
            au3 = attn_units(NSQ - 1)
            for u in au3[:2]:
                u()
            interleave(ou_all, au3[2:], frac=0.45)

    nc.compile()
    return nc


def _prep_inputs(x, wq, wk, wv, wo, freqs_cos, freqs_sin, mask):
    bf16 = ml_dtypes.bfloat16
    x2 = np.asarray(x, dtype=np.float32).reshape(S, D)
    xT = np.ascontiguousarray(x2.T).astype(bf16)
    cosT = np.repeat(np.asarray(freqs_cos, np.float32).T, 2, axis=0)
    sinT = np.repeat(np.asarray(freqs_sin, np.float32).T, 2, axis=0).copy()
    sinT[0::2] *= -1.0
    cosT = np.ascontiguousarray(cosT).astype(bf16)
    sinT = np.ascontiguousarray(sinT).astype(bf16)
    m2 = np.asarray(mask, np.float32).reshape(S, S)
    masks = np.stack([np.ascontiguousarray(m2[0:SQT, r * 128:(r + 1) * 128].T)
                      for r in range(4)]).astype(bf16)  # [4, 128, 512]
    in_maps = []
    for c in range(N_CORES):
        cols = slice(c * CW, (c + 1) * CW)
        in_maps.append({
            "xT": xT,
            "wq": np.ascontiguousarray(np.asarray(wq, np.float32)[:, cols]).astype(bf16),
            "wk": np.ascontiguousarray(np.asarray(wk, np.float32)[:, cols]).astype(bf16),
            "wv": np.ascontiguousarray(np.asarray(wv, np.float32)[:, cols]).astype(bf16),
            "wo": np.ascontiguousarray(np.asarray(wo, np.float32)[:, cols]).astype(bf16),
            "cosT": cosT,
            "ones": np.ones((HD, 1), bf16),
            "sinT": sinT,
            "masks": masks,
        })
    return in_maps


def kernel(x, wq, wk, wv, wo, freqs_cos, freqs_sin, mask):
    global LAST_RESULT
    from concourse.bass_utils import run_bass_kernel_spmd

    if "nc" not in _CACHE:
        _CACHE["nc"] = _build()
    nc = _CACHE["nc"]
    in_maps = _prep_inputs(x, wq, wk, wv, wo, freqs_cos, freqs_sin, mask)
    res = run_bass_kernel_spmd(nc, in_maps, core_ids=list(range(N_CORES)))
    LAST_RESULT = res
    out = np.concatenate([res.results[c]["out"].astype(np.float32)
                          for c in range(N_CORES)], axis=1)
    return out.reshape(B, S, D)


# revision 21
# speedup vs baseline: 1.0293x; 1.0293x over previous
"""Trainium2 Bass kernel for a LLaMA-style causal attention block.

Sharding (8 NeuronCores, one trn2 chip):
  - Tensor-parallel over heads: core c owns heads [4c, 4c+4) -> wq/wk/wv column
    slices [4096, 512]; computes qT/kT/v + RoPE + causal attention for its heads.
  - attnT [512, 2048] (bf16) is AllGather'd per sq quarter -> each core computes
    out[:, 512c:512c+512] = attn @ wo_cols.  Host concatenates column slices.

Layout trick: everything is computed transposed ([head_dim, seq]) so no
on-device transposes are needed:
  qT/kT = w_h.T @ xT      (xT host-pretransposed)
  scoresT[sk, sq] = kT_tile.T @ qT
  attnT[hd, sq] = v_tile.T @ expT
  out[sq, cols] = attnT_full_tile.T @ wo_tile
exp() needs no max-subtraction: scores are O(1) by construction.

v3 structure (vs v2):
  - trace showed PE at 92% busy but clocked 13/16 (GPIO power throttle) with
    three gaps (strip0 PSUM waits, tail AllGather exposure) each also causing
    a cold 4/8 re-throttle.  All v3 changes remove PE idle or PE cycles:
  - wk preloaded at prologue on the ACT queue (pass B was DMA-starved).
  - strip 0: pass B accumulates in the attention PSUM pool (idle in strip 0)
    so it starts while pass A's rope evacuations drain -> no PSUM-wait gap.
  - softmax denominator: ALL exp blocks accumulated on DVE into an f32 tile,
    ONE ones-matmul per (head, quarter) (v2 ran 5 ones-matmuls = 41k wasted
    PE cycles).  Mask adds trimmed to the 128 columns that are actually
    masked.
  - epilogue: outproj q0-q3 emitted as one primary stream; attention of the
    last quarter drains inside the first quarter (frac 0.25) so its
    AllGather (~47us incl. cross-core skew) completes long before outproj q3
    needs it.  q3's last 4 contraction tiles run ss-major with per-ss stores
    to stagger the final store tail.

Compute dtype bf16 (f32 PSUM accumulation), I/O f32.
"""

import math
import os
import sys

for _p in ("/opt/trn_rl_repo",):
    if os.path.isdir(_p) and _p not in sys.path:
        sys.path.insert(0, _p)

import numpy as np
import ml_dtypes

N_CORES = 8
B, S, D, H = 1, 2048, 4096, 32
HD = D // H          # 128
HPC = H // N_CORES   # 4 heads per core
CW = D // N_CORES    # 512 columns per core
NK = D // 128        # 32 contraction tiles
SQT = 512            # sq tile width
NSQ = S // SQT       # 4
SCALE = 1.0 / math.sqrt(HD)

_CACHE = {}
LAST_RESULT = None   # test harness reads exec_time_ns from here


def _build():
    import concourse.mybir as mybir
    import concourse.tile as tile
    from concourse import bacc
    from concourse.bass_isa import ReduceOp

    dt = mybir.dt
    f32, bf16 = dt.float32, dt.bfloat16

    nc = bacc.Bacc("TRN2", target_bir_lowering=False, debug=False,
                   num_devices=N_CORES)

    xT = nc.dram_tensor("xT", [D, S], bf16, kind="ExternalInput").ap()
    wq = nc.dram_tensor("wq", [D, CW], bf16, kind="ExternalInput").ap()
    wk = nc.dram_tensor("wk", [D, CW], bf16, kind="ExternalInput").ap()
    wv = nc.dram_tensor("wv", [D, CW], bf16, kind="ExternalInput").ap()
    wo = nc.dram_tensor("wo", [D, CW], bf16, kind="ExternalInput").ap()
    cosT = nc.dram_tensor("cosT", [HD, S], bf16, kind="ExternalInput").ap()
    sinT = nc.dram_tensor("sinT", [HD, S], bf16, kind="ExternalInput").ap()
    ones = nc.dram_tensor("ones", [HD, 1], bf16, kind="ExternalInput").ap()
    masks = nc.dram_tensor("masks", [4, 128, SQT], bf16, kind="ExternalInput").ap()
    out = nc.dram_tensor("out", [S, CW], bf16, kind="ExternalOutput").ap()

    swap_mask = []
    for i in range(16):
        swap_mask += [2 * i + 1, 2 * i]

    rg = [list(range(N_CORES))]

    with tile.TileContext(nc) as tc:
        with (
            tc.tile_pool(name="consts", bufs=1) as cpool,
            tc.tile_pool(name="wqp", bufs=NK) as wqp,    # wq resident; reused by wo
            tc.tile_pool(name="wkp", bufs=NK) as wkp,    # wk resident; reused by ag
            tc.tile_pool(name="wvp", bufs=NK) as wvp,    # wv resident
            tc.tile_pool(name="xp", bufs=33) as xpool,   # x strip ring
            tc.tile_pool(name="res", bufs=1) as res,     # qrot/krot/v_sb
            tc.tile_pool(name="rope", bufs=2) as ropep,
            tc.tile_pool(name="expp", bufs=4) as expp,
            tc.tile_pool(name="accp", bufs=1) as accp,
            tc.tile_pool(name="nrm", bufs=1) as nrm,
            tc.tile_pool(name="attnsb", bufs=2) as attnsb,
            tc.tile_pool(name="psq", bufs=4, space="PSUM") as psq,
            tc.tile_pool(name="psa", bufs=4, space="PSUM") as psa,
            tc.tile_pool(name="dram", bufs=1, space="DRAM") as dram,
        ):
            # resident results of QKV+rope
            qrot = [res.tile([HD, S], bf16, name=f"qrot{h}") for h in range(HPC)]
            krot = [res.tile([HD, S], bf16, name=f"krot{h}") for h in range(HPC)]
            v_sb = [res.tile([128, CW], bf16, name=f"v{i}") for i in range(S // 128)]

            # AllGather bounce buffers (one per sq quarter)
            ag_in = [dram.tile([HPC * HD, SQT], bf16, name=f"agin{q}")
                     for q in range(NSQ)]
            ag_out = [dram.tile([D, SQT], bf16, addr_space="Shared",
                                name=f"agout{q}") for q in range(NSQ)]

            cos_sb = cpool.tile([HD, S], bf16, name="cos_sb")
            sin_sb = cpool.tile([HD, S], bf16, name="sin_sb")
            ones_sb = cpool.tile([HD, 1], bf16, name="ones_sb")
            mask_sb = [cpool.tile([128, SQT], bf16, name=f"mask{r}")
                       for r in range(4)]

            # ---- resident weights, loaded once on parallel queues ----
            # (only SP/Activation/gpsimd can issue DMAs.  HBM demand in the
            # first ~45us is the constraint: wq+x must land for pass A, so
            # wq rides gpsimd and x strip 0 splits across SP/Activation.
            # Everything else queues BEHIND those transfers on the same DMA
            # queues so the hardware self-staggers it: scalar queue carries
            # x-odds -> consts -> wk (wk needed from ~50us), gpsimd carries
            # wq -> wv (wv needed from ~80us).)
            wq_sb, wk_sb, wv_sb = [], [], []
            for d in range(NK):
                wqt = wqp.tile([128, CW], bf16, tag="wq", name=f"wq{d}")
                nc.gpsimd.dma_start(wqt[:], wq[d * 128:(d + 1) * 128, :])
                wq_sb.append(wqt)

            wo_sb = []   # filled during strip 3 (reuses wqp slots)

            def emit_rope(pst, rot, sq0):
                # rot = t*cos + shuffle(t)*sin'   (sin' sign-baked)
                tbf = ropep.tile([128, SQT], bf16, tag="rbf", name="rbf")
                nc.scalar.copy(tbf[:], pst[:])          # frees the PSUM bank
                tsw = ropep.tile([128, SQT], bf16, tag="rsw", name="rsw")
                nc.vector.stream_shuffle(tsw[:], tbf[:], swap_mask)
                nc.vector.tensor_mul(tbf[:], tbf[:], cos_sb[:, sq0:sq0 + SQT])
                nc.vector.tensor_mul(tsw[:], tsw[:], sin_sb[:, sq0:sq0 + SQT])
                nc.vector.tensor_add(rot[:, sq0:sq0 + SQT], tbf[:], tsw[:])

            def strip_units(st):
                """Strip st as a list of emit units (~1us of PE work each):
                A: q heads 0-3 (streams x+wq), B: k heads 0-3, C: v. One
                PSUM bank per accumulator; in strip 0 pass B borrows the
                (idle) attention pool so it starts while pass A's rope
                evacuations drain; elsewhere interleaved attention work
                gives the ACT engine time to free banks between passes."""
                sq0 = st * SQT
                units = []
                x_tiles = []
                st_state = {}

                def u_head():
                    for d in range(NK):
                        xt = xpool.tile([128, SQT], bf16, tag="x",
                                        name=f"x{st}_{d}")
                        eng = nc.scalar if (st == 0 and d % 2) else nc.sync
                        eng.dma_start(xt[:], xT[d * 128:(d + 1) * 128,
                                                sq0:sq0 + SQT])
                        x_tiles.append(xt)
                    if st == 0:
                        # only x + wq fit in the first ~45us of HBM (8 cores
                        # contend); wk/wv/consts queue BEHIND the x streams on
                        # their engines' DMA rings so the hardware staggers
                        # them into 30-75us, just ahead of pass B/C/rope use.
                        for d in range(NK):
                            wkt = wkp.tile([128, CW], bf16, tag="wk",
                                           name=f"wk{d}")
                            nc.scalar.dma_start(wkt[:],
                                                wk[d * 128:(d + 1) * 128, :])
                            wk_sb.append(wkt)
                        nc.scalar.dma_start(cos_sb[:], cosT[:])
                        nc.scalar.dma_start(sin_sb[:], sinT[:])
                        nc.scalar.dma_start(ones_sb[:], ones[:])
                        for r in range(4):
                            nc.scalar.dma_start(mask_sb[r][:], masks[r])
                        for d in range(NK):
                            wvt = wvp.tile([128, CW], bf16, tag="wv",
                                           name=f"wv{d}")
                            nc.sync.dma_start(wvt[:],
                                              wv[d * 128:(d + 1) * 128, :])
                            wv_sb.append(wvt)
                units.append(u_head)

                for pi, wname in enumerate(("wq", "wk")):
                    def u_pre(pi=pi):
                        if st == 0 and pi == 1:
                            pool, tg = psa, "c"
                        else:
                            pool, tg = psq, "b"
                        st_state[pi] = [pool.tile([128, SQT], f32, tag=tg,
                                                  name=f"qk{st}_{pi}_{h}")
                                        for h in range(HPC)]
                    units.append(u_pre)
                    for d in range(NK):
                        def u_d(d=d, pi=pi):
                            wsb = wq_sb if pi == 0 else wk_sb
                            first, last = d == 0, d == NK - 1
                            xt = x_tiles[d]
                            for h in range(HPC):
                                nc.tensor.matmul(
                                    st_state[pi][h][:],
                                    wsb[d][:, h * HD:(h + 1) * HD],
                                    xt[:], start=first, stop=last)
                        units.append(u_d)
                    for h in range(HPC):
                        def u_rope(h=h, pi=pi):
                            rots = qrot if pi == 0 else krot
                            emit_rope(st_state[pi][h], rots[h], sq0)
                        units.append(u_rope)
                    if st == 3 and pi == 0:
                        def u_wo():
                            for d in range(NK):
                                wot = wqp.tile([128, CW], bf16, tag="wq",
                                               name=f"wo{d}")
                                nc.gpsimd.dma_start(
                                    wot[:], wo[d * 128:(d + 1) * 128, :])
                                wo_sb.append(wot)
                        units.append(u_wo)

                def u_vpre():
                    st_state["v"] = [psq.tile([128, CW], f32, tag="b",
                                              name=f"vps{st}_{ss}")
                                     for ss in range(4)]
                units.append(u_vpre)
                for d in range(NK):
                    def u_vd(d=d):
                        first, last = d == 0, d == NK - 1
                        for ss in range(4):
                            nc.tensor.matmul(
                                st_state["v"][ss][:],
                                x_tiles[d][:, ss * 128:(ss + 1) * 128],
                                wv_sb[d][:], start=first, stop=last)
                    units.append(u_vd)
                def u_vcopy():
                    for ss in range(4):
                        nc.scalar.copy(v_sb[st * 4 + ss][:],
                                       st_state["v"][ss][:])
                units.append(u_vcopy)
                return units

            def attn_units(sqT):
                """Attention for quarter sqT as emit units, interleaved into
                the following strip (or the output projection for sqT=3) so
                its exp/DVE load overlaps foreign matmul streams."""
                sq0 = sqT * SQT
                nblk = 4 * (sqT + 1)
                units = []
                for h in range(HPC):
                    hs = {}
                    exp_tiles = []

                    def emit_pv(j, hs=hs, h=h, exp_tiles=exp_tiles, nblk=nblk):
                        first, last = j == 0, j == nblk - 1
                        e, off = exp_tiles[j]
                        n = SQT - off
                        nc.tensor.matmul(hs["attn"][:, off:SQT],
                                         v_sb[j][:, h * HD:(h + 1) * HD],
                                         e[:, 0:n],
                                         start=first, stop=last)

                    for i in range(nblk):
                        def u_blk(i=i, h=h, hs=hs, exp_tiles=exp_tiles,
                                  nblk=nblk, emit_pv=emit_pv):
                            if i == 0:
                                hs["attn"] = psa.tile([HD, SQT], f32, tag="c",
                                                      name=f"aps{sqT}_{h}")
                                hs["acc"] = accp.tile(
                                    [128, SQT], f32, tag="acc",
                                    name=f"acc{sqT}_{h}")
                                hs["accbf"] = accp.tile(
                                    [128, SQT], bf16, tag="accbf",
                                    bufs=1, name=f"accbf{sqT}_{h}")
                            r = i - 4 * sqT
                            off = max(0, r) * 128
                            n = SQT - off
                            sc = psa.tile([128, SQT], f32, tag="c",
                                          name=f"sc{sqT}_{h}_{i}")
                            nc.tensor.matmul(sc[:, 0:n],
                                             krot[h][:, i * 128:(i + 1) * 128],
                                             qrot[h][:, sq0 + off:sq0 + SQT],
                                             start=True, stop=True)
                            if r >= 0:
                                # only the first 128 columns of the block can
                                # contain masked (query < key) elements
                                w = min(128, n)
                                nc.vector.tensor_add(sc[:, 0:w], sc[:, 0:w],
                                                     mask_sb[r][:, off:off + w])
                            e = expp.tile([128, SQT], bf16, tag="e",
                                          name=f"e{sqT}_{h}_{i}")
                            nc.scalar.activation(
                                e[:, 0:n], sc[:, 0:n],
                                mybir.ActivationFunctionType.Exp, scale=SCALE)
                            if i == 0:
                                nc.vector.tensor_copy(hs["acc"][:], e[:])
                            else:
                                nc.vector.tensor_add(hs["acc"][:, off:SQT],
                                                     hs["acc"][:, off:SQT],
                                                     e[:, 0:n])
                            if i == nblk - 1:
                                nc.vector.tensor_copy(hs["accbf"][:],
                                                      hs["acc"][:])
                            exp_tiles.append((e, off))
                            if i >= 2:
                                emit_pv(i - 2)
                        units.append(u_blk)

                    def u_tail(h=h, hs=hs, exp_tiles=exp_tiles, nblk=nblk,
                               emit_pv=emit_pv):
                        emit_pv(nblk - 2)
                        emit_pv(nblk - 1)
                        den_ps = psa.tile([1, SQT], f32, tag="c",
                                          name=f"dps{sqT}_{h}")
                        hs["den"] = den_ps
                        nc.tensor.matmul(den_ps[:], ones_sb[:],
                                         hs["accbf"][:],
                                         start=True, stop=True)
                    units.append(u_tail)

                    def u_norm(h=h, hs=hs):
                        rec = nrm.tile([1, SQT], f32, tag="rec",
                                       name=f"rec{sqT}_{h}")
                        nc.vector.reciprocal_approx_fast(out=rec[:],
                                                         in_=hs["den"][:])
                        araw = attnsb.tile([HD, SQT], bf16, tag="a",
                                           name=f"araw{sqT}_{h}")
                        nc.vector.tensor_copy(araw[:], hs["attn"][:])
                        bc = nrm.tile([128, SQT], f32, tag="bc",
                                      name=f"bc{sqT}_{h}")
                        nc.gpsimd.partition_broadcast(bc[:], rec[:],
                                                      channels=128)
                        a_sb = attnsb.tile([HD, SQT], bf16, tag="a",
                                           name=f"asb{sqT}_{h}")
                        nc.vector.tensor_mul(a_sb[:], araw[:], bc[:])
                        nc.gpsimd.dma_start(
                            ag_in[sqT][h * HD:(h + 1) * HD, :], a_sb[:])
                    units.append(u_norm)

                def u_ag():
                    nc.gpsimd.collective_compute(
                        "AllGather", mybir.AluOpType.bypass, replica_groups=rg,
                        ins=[ag_in[sqT].opt()], outs=[ag_out[sqT].opt()])
                units.append(u_ag)
                return units

            def outproj_units(q):
                units = []
                qs = {}
                nd_main = NK if q < 3 else NK - 4
                for d in range(nd_main):
                    def u_od(d=d, q=q, qs=qs):
                        if d == 0:
                            qs["o"] = [psq.tile([128, CW], f32, tag="b",
                                                name=f"ops{q}_{ss}")
                                       for ss in range(4)]
                        agt = wkp.tile([128, SQT], bf16, tag="wk",
                                       name=f"agt{q}_{d}")
                        eng = nc.gpsimd if d % 2 else nc.sync
                        eng.dma_start(agt[:],
                                      ag_out[q][d * 128:(d + 1) * 128, :])
                        first, last = d == 0, d == NK - 1
                        for ss in range(4):
                            nc.tensor.matmul(
                                qs["o"][ss][:],
                                agt[:, ss * 128:(ss + 1) * 128],
                                wo_sb[d][:], start=first, stop=last)
                    units.append(u_od)
                if q < 3:
                    def u_ost(q=q, qs=qs):
                        # stores ride the ACT queue: gpsimd must stay free
                        # for the attn3 norm broadcasts + ag_in DMAs
                        for ss in range(4):
                            o = attnsb.tile([128, CW], bf16, tag="a",
                                            name=f"o{q}_{ss}")
                            nc.scalar.copy(o[:], qs["o"][ss][:])
                            nc.scalar.dma_start(
                                out[q * SQT + ss * 128:q * SQT + (ss + 1) * 128,
                                    :], o[:])
                    units.append(u_ost)
                else:
                    # last 4 contraction tiles ss-major with per-ss stores so
                    # the final store tail staggers instead of arriving at once
                    def u_ldtail(qs=qs):
                        qs["agt_tail"] = []
                        for d in range(NK - 4, NK):
                            agt = wkp.tile([128, SQT], bf16, tag="wk",
                                           name=f"agt3_{d}")
                            eng = nc.gpsimd if d % 2 else nc.sync
                            eng.dma_start(
                                agt[:], ag_out[3][d * 128:(d + 1) * 128, :])
                            qs["agt_tail"].append(agt)
                    units.append(u_ldtail)
                    for ss in range(4):
                        def u_fin(ss=ss, qs=qs):
                            for j, d in enumerate(range(NK - 4, NK)):
                                nc.tensor.matmul(
                                    qs["o"][ss][:],
                                    qs["agt_tail"][j][:, ss * 128:(ss + 1) * 128],
                                    wo_sb[d][:], start=False, stop=(d == NK - 1))
                            o = attnsb.tile([128, CW], bf16, tag="a",
                                            name=f"o3_{ss}")
                            nc.scalar.copy(o[:], qs["o"][ss][:])
                            nc.sync.dma_start(
                                out[3 * SQT + ss * 128:3 * SQT + (ss + 1) * 128,
                                    :], o[:])
                        units.append(u_fin)
                return units

            def interleave(primary, secondary, frac=1.0):
                # drain `secondary` within the first `frac` of `primary`
                n, m = len(primary), len(secondary)
                j = 0
                for i, u in enumerate(primary):
                    u()
                    target = min(m, int((i + 1) * m / (n * frac)))
                    while j < target:
                        secondary[j]()
                        j += 1
                while j < m:
                    secondary[j]()
                    j += 1

            for st in range(NSQ):
                su = strip_units(st)
                au = attn_units(st - 1) if st >= 1 else []
                interleave(su, au)
            # attention of the last strip drains inside outproj q0 so its
            # AllGather (incl. cross-core skew wait) hides under q1-q2
            ou_all = []
            for q in range(NSQ):
                ou_all += outproj_units(q)
            au3 = attn_units(NSQ - 1)
            for u in au3[:2]:
                u()
            interleave(ou_all, au3[2:], frac=0.27)

    nc.compile()
    return nc


def _prep_inputs(x, wq, wk, wv, wo, freqs_cos, freqs_sin, mask):
    bf16 = ml_dtypes.bfloat16
    x2 = np.asarray(x, dtype=np.float32).reshape(S, D)
    xT = np.ascontiguousarray(x2.T).astype(bf16)
    cosT = np.repeat(np.asarray(freqs_cos, np.float32).T, 2, axis=0)
    sinT = np.repeat(np.asarray(freqs_sin, np.float32).T, 2, axis=0).copy()
    sinT[0::2] *= -1.0
    cosT = np.ascontiguousarray(cosT).astype(bf16)
    sinT = np.ascontiguousarray(sinT).astype(bf16)
    m2 = np.asarray(mask, np.float32).reshape(S, S)
    masks = np.stack([np.ascontiguousarray(m2[0:SQT, r * 128:(r + 1) * 128].T)
                      for r in range(4)]).astype(bf16)  # [4, 128, 512]
    in_maps = []
    for c in range(N_CORES):
        cols = slice(c * CW, (c + 1) * CW)
        in_maps.append({
            "xT": xT,
            "wq": np.ascontiguousarray(np.asarray(wq, np.float32)[:, cols]).astype(bf16),
            "wk": np.ascontiguousarray(np.asarray(wk, np.float32)[:, cols]).astype(bf16),
            "wv": np.ascontiguousarray(np.asarray(wv, np.float32)[:, cols]).astype(bf16),
            "wo": np.ascontiguousarray(np.asarray(wo, np.float32)[:, cols]).astype(bf16),
            "cosT": cosT,
            "ones": np.ones((HD, 1), bf16),
            "sinT": sinT,
            "masks": masks,
        })
    return in_maps


def kernel(x, wq, wk, wv, wo, freqs_cos, freqs_sin, mask):
    global LAST_RESULT
    from concourse.bass_utils import run_bass_kernel_spmd

    if "nc" not in _CACHE:
        _CACHE["nc"] = _build()
    nc = _CACHE["nc"]
    in_maps = _prep_inputs(x, wq, wk, wv, wo, freqs_cos, freqs_sin, mask)
    res = run_bass_kernel_spmd(nc, in_maps, core_ids=list(range(N_CORES)))
    LAST_RESULT = res
    out = np.concatenate([res.results[c]["out"].astype(np.float32)
                          for c in range(N_CORES)], axis=1)
    return out.reshape(B, S, D)


# revision 26
# speedup vs baseline: 1.0314x; 1.0021x over previous
"""Trainium2 Bass kernel for a LLaMA-style causal attention block.

Sharding (8 NeuronCores, one trn2 chip):
  - Tensor-parallel over heads: core c owns heads [4c, 4c+4) -> wq/wk/wv column
    slices [4096, 512]; computes qT/kT/v + RoPE + causal attention for its heads.
  - attnT [512, 2048] (bf16) is AllGather'd per sq quarter -> each core computes
    out[:, 512c:512c+512] = attn @ wo_cols.  Host concatenates column slices.

Layout trick: everything is computed transposed ([head_dim, seq]) so no
on-device transposes are needed:
  qT/kT = w_h.T @ xT      (xT host-pretransposed)
  scoresT[sk, sq] = kT_tile.T @ qT
  attnT[hd, sq] = v_tile.T @ expT
  out[sq, cols] = attnT_full_tile.T @ wo_tile
exp() needs no max-subtraction: scores are O(1) by construction.

v3 structure (vs v2):
  - trace showed PE at 92% busy but clocked 13/16 (GPIO power throttle) with
    three gaps (strip0 PSUM waits, tail AllGather exposure) each also causing
    a cold 4/8 re-throttle.  All v3 changes remove PE idle or PE cycles:
  - wk preloaded at prologue on the ACT queue (pass B was DMA-starved).
  - strip 0: pass B accumulates in the attention PSUM pool (idle in strip 0)
    so it starts while pass A's rope evacuations drain -> no PSUM-wait gap.
  - softmax denominator: ALL exp blocks accumulated on DVE into an f32 tile,
    ONE ones-matmul per (head, quarter) (v2 ran 5 ones-matmuls = 41k wasted
    PE cycles).  Mask adds trimmed to the 128 columns that are actually
    masked.
  - epilogue: outproj q0-q3 emitted as one primary stream; attention of the
    last quarter drains inside the first quarter (frac 0.25) so its
    AllGather (~47us incl. cross-core skew) completes long before outproj q3
    needs it.  q3's last 4 contraction tiles run ss-major with per-ss stores
    to stagger the final store tail.

Compute dtype bf16 (f32 PSUM accumulation), I/O f32.
"""

import math
import os
import sys

for _p in ("/opt/trn_rl_repo",):
    if os.path.isdir(_p) and _p not in sys.path:
        sys.path.insert(0, _p)

import numpy as np
import ml_dtypes

N_CORES = 8
B, S, D, H = 1, 2048, 4096, 32
HD = D // H          # 128
HPC = H // N_CORES   # 4 heads per core
CW = D // N_CORES    # 512 columns per core
NK = D // 128        # 32 contraction tiles
SQT = 512            # sq tile width
NSQ = S // SQT       # 4
SCALE = 1.0 / math.sqrt(HD)

_CACHE = {}
LAST_RESULT = None   # test harness reads exec_time_ns from here


def _build():
    import concourse.mybir as mybir
    import concourse.tile as tile
    from concourse import bacc
    from concourse.bass_isa import ReduceOp

    dt = mybir.dt
    f32, bf16 = dt.float32, dt.bfloat16

    nc = bacc.Bacc("TRN2", target_bir_lowering=False, debug=False,
                   num_devices=N_CORES)

    xT = nc.dram_tensor("xT", [D, S], bf16, kind="ExternalInput").ap()
    wq = nc.dram_tensor("wq", [D, CW], bf16, kind="ExternalInput").ap()
    wk = nc.dram_tensor("wk", [D, CW], bf16, kind="ExternalInput").ap()
    wv = nc.dram_tensor("wv", [D, CW], bf16, kind="ExternalInput").ap()
    wo = nc.dram_tensor("wo", [D, CW], bf16, kind="ExternalInput").ap()
    cosT = nc.dram_tensor("cosT", [HD, S], bf16, kind="ExternalInput").ap()
    sinT = nc.dram_tensor("sinT", [HD, S], bf16, kind="ExternalInput").ap()
    ones = nc.dram_tensor("ones", [HD, 1], bf16, kind="ExternalInput").ap()
    masks = nc.dram_tensor("masks", [4, 128, SQT], bf16, kind="ExternalInput").ap()
    out = nc.dram_tensor("out", [S, CW], bf16, kind="ExternalOutput").ap()

    swap_mask = []
    for i in range(16):
        swap_mask += [2 * i + 1, 2 * i]

    rg = [list(range(N_CORES))]

    with tile.TileContext(nc) as tc:
        with (
            tc.tile_pool(name="consts", bufs=1) as cpool,
            tc.tile_pool(name="wqp", bufs=NK) as wqp,    # wq resident; reused by wo
            tc.tile_pool(name="wkp", bufs=NK) as wkp,    # wk resident; reused by ag
            tc.tile_pool(name="wvp", bufs=NK) as wvp,    # wv resident
            tc.tile_pool(name="xp", bufs=32) as xpool,   # x strip ring
            tc.tile_pool(name="res", bufs=1) as res,     # qrot/krot/v_sb
            tc.tile_pool(name="rope", bufs=3) as ropep,
            tc.tile_pool(name="expp", bufs=4) as expp,
            tc.tile_pool(name="accp", bufs=1) as accp,
            tc.tile_pool(name="nrm", bufs=1) as nrm,
            tc.tile_pool(name="attnsb", bufs=2) as attnsb,
            tc.tile_pool(name="psq", bufs=4, space="PSUM") as psq,
            tc.tile_pool(name="psa", bufs=4, space="PSUM") as psa,
            tc.tile_pool(name="dram", bufs=1, space="DRAM") as dram,
        ):
            # resident results of QKV+rope
            qrot = [res.tile([HD, S], bf16, name=f"qrot{h}") for h in range(HPC)]
            krot = [res.tile([HD, S], bf16, name=f"krot{h}") for h in range(HPC)]
            v_sb = [res.tile([128, CW], bf16, name=f"v{i}") for i in range(S // 128)]

            # AllGather bounce buffers (one per sq quarter)
            ag_in = [dram.tile([HPC * HD, SQT], bf16, name=f"agin{q}")
                     for q in range(NSQ)]
            ag_out = [dram.tile([D, SQT], bf16, addr_space="Shared",
                                name=f"agout{q}") for q in range(NSQ)]

            cos_sb = cpool.tile([HD, S], bf16, name="cos_sb")
            sin_sb = cpool.tile([HD, S], bf16, name="sin_sb")
            ones_sb = cpool.tile([HD, 1], bf16, name="ones_sb")
            mask_sb = [cpool.tile([128, SQT], bf16, name=f"mask{r}")
                       for r in range(4)]

            # ---- resident weights, loaded once on parallel queues ----
            # (only SP/Activation/gpsimd can issue DMAs.  HBM demand in the
            # first ~45us is the constraint: wq+x must land for pass A, so
            # wq rides gpsimd and x strip 0 splits across SP/Activation.
            # Everything else queues BEHIND those transfers on the same DMA
            # queues so the hardware self-staggers it: scalar queue carries
            # x-odds -> consts -> wk (wk needed from ~50us), gpsimd carries
            # wq -> wv (wv needed from ~80us).)
            wq_sb, wk_sb, wv_sb = [], [], []
            # strip-0 x and wq are equally urgent (both stream into pass A
            # from ~11us): interleave them 3-ways so each engine's DMA rings
            # carry ~2.7MB in arrival order
            engs3 = [nc.sync, nc.scalar, nc.gpsimd]
            x0_tiles = []
            for d in range(NK):
                wqt = wqp.tile([128, CW], bf16, tag="wq", name=f"wq{d}")
                engs3[d % 3].dma_start(wqt[:], wq[d * 128:(d + 1) * 128, :])
                wq_sb.append(wqt)
                xt = xpool.tile([128, SQT], bf16, tag="x", name=f"x0_{d}")
                engs3[(d + 1) % 3].dma_start(xt[:],
                                             xT[d * 128:(d + 1) * 128, 0:SQT])
                x0_tiles.append(xt)

            wo_sb = []   # filled during strip 3 (reuses wqp slots)

            def emit_rope(pst, rot, sq0):
                # rot = t*cos + shuffle(t)*sin'   (sin' sign-baked)
                tbf = ropep.tile([128, SQT], bf16, tag="rbf", name="rbf")
                nc.scalar.copy(tbf[:], pst[:])          # frees the PSUM bank
                tsw = ropep.tile([128, SQT], bf16, tag="rsw", name="rsw")
                nc.vector.stream_shuffle(tsw[:], tbf[:], swap_mask)
                nc.vector.tensor_mul(tbf[:], tbf[:], cos_sb[:, sq0:sq0 + SQT])
                nc.vector.tensor_mul(tsw[:], tsw[:], sin_sb[:, sq0:sq0 + SQT])
                nc.vector.tensor_add(rot[:, sq0:sq0 + SQT], tbf[:], tsw[:])

            def strip_units(st):
                """Strip st as a list of emit units (~1us of PE work each):
                A: q heads 0-3 (streams x+wq), B: k heads 0-3, C: v. One
                PSUM bank per accumulator; in strip 0 pass B borrows the
                (idle) attention pool so it starts while pass A's rope
                evacuations drain; elsewhere interleaved attention work
                gives the ACT engine time to free banks between passes."""
                sq0 = st * SQT
                units = []
                x_tiles = []
                st_state = {}

                def u_head():
                    if st == 0:
                        # x strip 0 was already issued in the prologue,
                        # interleaved with wq.  Everything else queues BEHIND
                        # those transfers on the engines' DMA rings so the
                        # hardware staggers it into 35-75us, just ahead of
                        # pass B/C/rope use.
                        x_tiles.extend(x0_tiles)
                        for d in range(NK):
                            wkt = wkp.tile([128, CW], bf16, tag="wk",
                                           name=f"wk{d}")
                            eng = nc.scalar if d % 2 == 0 else nc.gpsimd
                            eng.dma_start(wkt[:],
                                          wk[d * 128:(d + 1) * 128, :])
                            wk_sb.append(wkt)
                        nc.scalar.dma_start(cos_sb[:], cosT[:])
                        nc.scalar.dma_start(sin_sb[:], sinT[:])
                        nc.scalar.dma_start(ones_sb[:], ones[:])
                        for r in range(4):
                            nc.scalar.dma_start(mask_sb[r][:], masks[r])
                        for d in range(NK):
                            wvt = wvp.tile([128, CW], bf16, tag="wv",
                                           name=f"wv{d}")
                            nc.sync.dma_start(wvt[:],
                                              wv[d * 128:(d + 1) * 128, :])
                            wv_sb.append(wvt)
                    else:
                        for d in range(NK):
                            xt = xpool.tile([128, SQT], bf16, tag="x",
                                            name=f"x{st}_{d}")
                            nc.sync.dma_start(xt[:],
                                              xT[d * 128:(d + 1) * 128,
                                                 sq0:sq0 + SQT])
                            x_tiles.append(xt)
                units.append(u_head)

                for pi, wname in enumerate(("wq", "wk")):
                    def u_pre(pi=pi):
                        if st == 0 and pi == 1:
                            pool, tg = psa, "c"
                        else:
                            pool, tg = psq, "b"
                        st_state[pi] = [pool.tile([128, SQT], f32, tag=tg,
                                                  name=f"qk{st}_{pi}_{h}")
                                        for h in range(HPC)]
                    units.append(u_pre)
                    for d in range(NK):
                        def u_d(d=d, pi=pi):
                            wsb = wq_sb if pi == 0 else wk_sb
                            first, last = d == 0, d == NK - 1
                            xt = x_tiles[d]
                            for h in range(HPC):
                                nc.tensor.matmul(
                                    st_state[pi][h][:],
                                    wsb[d][:, h * HD:(h + 1) * HD],
                                    xt[:], start=first, stop=last)
                        units.append(u_d)
                    for h in range(HPC):
                        def u_rope(h=h, pi=pi):
                            rots = qrot if pi == 0 else krot
                            emit_rope(st_state[pi][h], rots[h], sq0)
                        units.append(u_rope)
                    if st == 3 and pi == 0:
                        def u_wo():
                            for d in range(NK):
                                wot = wqp.tile([128, CW], bf16, tag="wq",
                                               name=f"wo{d}")
                                nc.gpsimd.dma_start(
                                    wot[:], wo[d * 128:(d + 1) * 128, :])
                                wo_sb.append(wot)
                        units.append(u_wo)

                def u_vpre():
                    st_state["v"] = [psq.tile([128, CW], f32, tag="b",
                                              name=f"vps{st}_{ss}")
                                     for ss in range(4)]
                units.append(u_vpre)
                for d in range(NK):
                    def u_vd(d=d):
                        first, last = d == 0, d == NK - 1
                        for ss in range(4):
                            nc.tensor.matmul(
                                st_state["v"][ss][:],
                                x_tiles[d][:, ss * 128:(ss + 1) * 128],
                                wv_sb[d][:], start=first, stop=last)
                    units.append(u_vd)
                def u_vcopy():
                    for ss in range(4):
                        nc.scalar.copy(v_sb[st * 4 + ss][:],
                                       st_state["v"][ss][:])
                units.append(u_vcopy)
                return units

            def attn_units(sqT):
                """Attention for quarter sqT as emit units, interleaved into
                the following strip (or the output projection for sqT=3) so
                its exp/DVE load overlaps foreign matmul streams."""
                sq0 = sqT * SQT
                nblk = 4 * (sqT + 1)
                units = []
                for h in range(HPC):
                    hs = {}
                    exp_tiles = []

                    def emit_pv(j, hs=hs, h=h, exp_tiles=exp_tiles, nblk=nblk):
                        first, last = j == 0, j == nblk - 1
                        e, off = exp_tiles[j]
                        n = SQT - off
                        nc.tensor.matmul(hs["attn"][:, off:SQT],
                                         v_sb[j][:, h * HD:(h + 1) * HD],
                                         e[:, 0:n],
                                         start=first, stop=last)

                    for i in range(nblk):
                        def u_blk(i=i, h=h, hs=hs, exp_tiles=exp_tiles,
                                  nblk=nblk, emit_pv=emit_pv):
                            if i == 0:
                                hs["attn"] = psa.tile([HD, SQT], f32, tag="c",
                                                      name=f"aps{sqT}_{h}")
                                hs["acc"] = accp.tile(
                                    [128, SQT], f32, tag="acc",
                                    name=f"acc{sqT}_{h}")
                                hs["accbf"] = accp.tile(
                                    [128, SQT], bf16, tag="accbf",
                                    bufs=1, name=f"accbf{sqT}_{h}")
                            r = i - 4 * sqT
                            off = max(0, r) * 128
                            n = SQT - off
                            sc = psa.tile([128, SQT], f32, tag="c",
                                          name=f"sc{sqT}_{h}_{i}")
                            nc.tensor.matmul(sc[:, 0:n],
                                             krot[h][:, i * 128:(i + 1) * 128],
                                             qrot[h][:, sq0 + off:sq0 + SQT],
                                             start=True, stop=True)
                            if r >= 0:
                                # only the first 128 columns of the block can
                                # contain masked (query < key) elements
                                w = min(128, n)
                                nc.vector.tensor_add(sc[:, 0:w], sc[:, 0:w],
                                                     mask_sb[r][:, off:off + w])
                            e = expp.tile([128, SQT], bf16, tag="e",
                                          name=f"e{sqT}_{h}_{i}")
                            nc.scalar.activation(
                                e[:, 0:n], sc[:, 0:n],
                                mybir.ActivationFunctionType.Exp, scale=SCALE)
                            if i == 0:
                                nc.vector.tensor_copy(hs["acc"][:], e[:])
                            else:
                                nc.vector.tensor_add(hs["acc"][:, off:SQT],
                                                     hs["acc"][:, off:SQT],
                                                     e[:, 0:n])
                            if i == nblk - 1:
                                nc.vector.tensor_copy(hs["accbf"][:],
                                                      hs["acc"][:])
                            exp_tiles.append((e, off))
                            if i >= 2:
                                emit_pv(i - 2)
                        units.append(u_blk)

                    def u_tail(h=h, hs=hs, exp_tiles=exp_tiles, nblk=nblk,
                               emit_pv=emit_pv):
                        emit_pv(nblk - 2)
                        emit_pv(nblk - 1)
                        den_ps = psa.tile([1, SQT], f32, tag="c",
                                          name=f"dps{sqT}_{h}")
                        hs["den"] = den_ps
                        nc.tensor.matmul(den_ps[:], ones_sb[:],
                                         hs["accbf"][:],
                                         start=True, stop=True)
                    units.append(u_tail)

                    def u_norm(h=h, hs=hs):
                        rec = nrm.tile([1, SQT], f32, tag="rec",
                                       name=f"rec{sqT}_{h}")
                        nc.vector.reciprocal_approx_fast(out=rec[:],
                                                         in_=hs["den"][:])
                        araw = attnsb.tile([HD, SQT], bf16, tag="a",
                                           name=f"araw{sqT}_{h}")
                        nc.vector.tensor_copy(araw[:], hs["attn"][:])
                        bc = nrm.tile([128, SQT], f32, tag="bc",
                                      name=f"bc{sqT}_{h}")
                        nc.gpsimd.partition_broadcast(bc[:], rec[:],
                                                      channels=128)
                        a_sb = attnsb.tile([HD, SQT], bf16, tag="a",
                                           name=f"asb{sqT}_{h}")
                        nc.vector.tensor_mul(a_sb[:], araw[:], bc[:])
                        nc.gpsimd.dma_start(
                            ag_in[sqT][h * HD:(h + 1) * HD, :], a_sb[:])
                    units.append(u_norm)

                def u_ag():
                    nc.gpsimd.collective_compute(
                        "AllGather", mybir.AluOpType.bypass, replica_groups=rg,
                        ins=[ag_in[sqT].opt()], outs=[ag_out[sqT].opt()])
                units.append(u_ag)
                return units

            def outproj_units(q):
                units = []
                qs = {}
                nd_main = NK if q < 3 else NK - 4
                for d in range(nd_main):
                    def u_od(d=d, q=q, qs=qs):
                        if d == 0:
                            qs["o"] = [psq.tile([128, CW], f32, tag="b",
                                                name=f"ops{q}_{ss}")
                                       for ss in range(4)]
                        agt = wkp.tile([128, SQT], bf16, tag="wk",
                                       name=f"agt{q}_{d}")
                        eng = nc.gpsimd if d % 2 else nc.sync
                        eng.dma_start(agt[:],
                                      ag_out[q][d * 128:(d + 1) * 128, :])
                        first, last = d == 0, d == NK - 1
                        for ss in range(4):
                            nc.tensor.matmul(
                                qs["o"][ss][:],
                                agt[:, ss * 128:(ss + 1) * 128],
                                wo_sb[d][:], start=first, stop=last)
                    units.append(u_od)
                if q < 3:
                    def u_ost(q=q, qs=qs):
                        # stores ride the ACT queue: gpsimd must stay free
                        # for the attn3 norm broadcasts + ag_in DMAs
                        for ss in range(4):
                            o = attnsb.tile([128, CW], bf16, tag="a",
                                            name=f"o{q}_{ss}")
                            nc.scalar.copy(o[:], qs["o"][ss][:])
                            nc.scalar.dma_start(
                                out[q * SQT + ss * 128:q * SQT + (ss + 1) * 128,
                                    :], o[:])
                    units.append(u_ost)
                else:
                    # last 4 contraction tiles ss-major with per-ss stores so
                    # the final store tail staggers instead of arriving at once
                    def u_ldtail(qs=qs):
                        qs["agt_tail"] = []
                        for d in range(NK - 4, NK):
                            agt = wkp.tile([128, SQT], bf16, tag="wk",
                                           name=f"agt3_{d}")
                            eng = nc.gpsimd if d % 2 else nc.sync
                            eng.dma_start(
                                agt[:], ag_out[3][d * 128:(d + 1) * 128, :])
                            qs["agt_tail"].append(agt)
                    units.append(u_ldtail)
                    for ss in range(4):
                        def u_fin(ss=ss, qs=qs):
                            for j, d in enumerate(range(NK - 4, NK)):
                                nc.tensor.matmul(
                                    qs["o"][ss][:],
                                    qs["agt_tail"][j][:, ss * 128:(ss + 1) * 128],
                                    wo_sb[d][:], start=False, stop=(d == NK - 1))
                            o = attnsb.tile([128, CW], bf16, tag="a",
                                            name=f"o3_{ss}")
                            nc.scalar.copy(o[:], qs["o"][ss][:])
                            nc.sync.dma_start(
                                out[3 * SQT + ss * 128:3 * SQT + (ss + 1) * 128,
                                    :], o[:])
                        units.append(u_fin)
                return units

            def interleave(primary, secondary, frac=1.0):
                # drain `secondary` within the first `frac` of `primary`
                n, m = len(primary), len(secondary)
                j = 0
                for i, u in enumerate(primary):
                    u()
                    target = min(m, int((i + 1) * m / (n * frac)))
                    while j < target:
                        secondary[j]()
                        j += 1
                while j < m:
                    secondary[j]()
                    j += 1

            for st in range(NSQ):
                su = strip_units(st)
                au = attn_units(st - 1) if st >= 1 else []
                interleave(su, au)
            # attention of the last strip drains inside outproj q0 so its
            # AllGather (incl. cross-core skew wait) hides under q1-q2
            ou_all = []
            for q in range(NSQ):
                ou_all += outproj_units(q)
            au3 = attn_units(NSQ - 1)
            for u in au3[:2]:
                u()
            interleave(ou_all, au3[2:], frac=0.27)

    nc.compile()
    return nc


def _prep_inputs(x, wq, wk, wv, wo, freqs_cos, freqs_sin, mask):
    bf16 = ml_dtypes.bfloat16
    x2 = np.asarray(x, dtype=np.float32).reshape(S, D)
    xT = np.ascontiguousarray(x2.T).astype(bf16)
    cosT = np.repeat(np.asarray(freqs_cos, np.float32).T, 2, axis=0)
    sinT = np.repeat(np.asarray(freqs_sin, np.float32).T, 2, axis=0).copy()
    sinT[0::2] *= -1.0
    cosT = np.ascontiguousarray(cosT).astype(bf16)
    sinT = np.ascontiguousarray(sinT).astype(bf16)
    m2 = np.asarray(mask, np.float32).reshape(S, S)
    masks = np.stack([np.ascontiguousarray(m2[0:SQT, r * 128:(r + 1) * 128].T)
                      for r in range(4)]).astype(bf16)  # [4, 128, 512]
    in_maps = []
    for c in range(N_CORES):
        cols = slice(c * CW, (c + 1) * CW)
        in_maps.append({
            "xT": xT,
            "wq": np.ascontiguousarray(np.asarray(wq, np.float32)[:, cols]).astype(bf16),
            "wk": np.ascontiguousarray(np.asarray(wk, np.float32)[:, cols]).astype(bf16),
            "wv": np.ascontiguousarray(np.asarray(wv, np.float32)[:, cols]).astype(bf16),
            "wo": np.ascontiguousarray(np.asarray(wo, np.float32)[:, cols]).astype(bf16),
            "cosT": cosT,
            "ones": np.ones((HD, 1), bf16),
            "sinT": sinT,
            "masks": masks,
        })
    return in_maps


def kernel(x, wq, wk, wv, wo, freqs_cos, freqs_sin, mask):
    global LAST_RESULT
    from concourse.bass_utils import run_bass_kernel_spmd

    if "nc" not in _CACHE:
        _CACHE["nc"] = _build()
    nc = _CACHE["nc"]
    in_maps = _prep_inputs(x, wq, wk, wv, wo, freqs_cos, freqs_sin, mask)
    res = run_bass_kernel_spmd(nc, in_maps, core_ids=list(range(N_CORES)))
    LAST_RESULT = res
    out = np.concatenate([res.results[c]["out"].astype(np.float32)
                          for c in range(N_CORES)], axis=1)
    return out.reshape(B, S, D)
